# revision 1
# baseline (speedup 1.0000x reference)
"""Trainium2 Bass kernel for nn_HSL1Loss (per-(batch,label) segment MSE loss).

loss = (1/B) * sum_b sum_{l=1..63, cnt>0} mean((feat[b][gt[b]==l] - l)^2)

Strategy: batch-data-parallel over 8 NeuronCores (2 images each). Per core,
each [128, N] tile is reduced into per-(batch,label) sum/count accumulators
with 64 fused mask-multiply-accumulate passes (scalar_tensor_tensor with
accum_out, bf16 2x mode) + 64 fused count passes (tensor_scalar is_equal with
accum_out, bf16 4x mode) on the Vector engine; the squared error is produced
on the Scalar engine. Partition reduce via GPSIMD, division + final reduction
on-device; host sums the 8 per-core partials (the scalar all-reduce).
"""
import numpy as np

import concourse.bass as bass
import concourse.bass_isa as bass_isa
import concourse.mybir as mybir
import concourse.tile as tile
from concourse.bass_utils import run_bass_kernel_spmd

# --- inline tile drain patch (kernel.py must be self-contained) -------------
from concourse import tile as _tile_mod


def _apply_drain_patch(max_waits=1):
    if getattr(_tile_mod.TileContext, "_drain_split_patched", False):
        return

    def _drain_and_barrier(self, tick_clock, wait_clock):
        drain_inst = self.nc.sync.drain()
        wait_clock.add_sem_waits(
            drain_inst.ins, _tile_mod.ScopedClock({None: tick_clock.global_clock})
        )
        si = drain_inst.ins.sync_info
        waits = list(si.on_wait or []) if si is not None else []
        if len(waits) > max_waits:
            upd = list(si.on_update or [])
            drain_inst.ins.sync_info = mybir.SyncInfo(
                on_wait=waits[:max_waits], on_update=upd
            )
            for i in range(max_waits, len(waits), max_waits):
                d2 = self.nc.sync.drain()
                d2.ins.sync_info = mybir.SyncInfo(
                    on_wait=waits[i : i + max_waits], on_update=[]
                )
        self.nc.all_engine_barrier()
        assert self.sems is not None
        popped = self.nc._tile_sem_poison_stack.pop()
        assert popped is self._sem_poison
        self.nc.clear_and_free_semaphores(list(self.sems.allocated().values()))
        self.nc.all_engine_barrier()

    _tile_mod.TileContext._drain_and_barrier = _drain_and_barrier
    _tile_mod.TileContext._drain_split_patched = True


_apply_drain_patch()

_MAX_INST_WAITS = 1
_wsplit_counter = [0]


def _split_waits(nc, k=_MAX_INST_WAITS):
    """Walrus in this toolchain rejects instructions with >k sem waits.
    Move excess waits onto same-engine NoOps inserted just before."""
    for fn in nc.m.functions:
        for bb in fn.blocks:
            il = list(bb.instructions)
            out = []
            changed = False
            for ins in il:
                si = ins.sync_info
                waits = list(si.on_wait or []) if si is not None else []
                if len(waits) > k:
                    changed = True
                    chunks = [waits[i : i + k] for i in range(0, len(waits), k)]
                    for ch in chunks[:-1]:
                        _wsplit_counter[0] += 1
                        nop = mybir.InstNoOp(
                            name=f"WSPLIT-{_wsplit_counter[0]}", ins=[], outs=[]
                        )
                        nop.engine = ins.engine
                        nop.sync_info = mybir.SyncInfo(on_wait=ch, on_update=[])
                        out.append(nop)
                    ins.sync_info = mybir.SyncInfo(
                        on_wait=chunks[-1], on_update=list(si.on_update or [])
                    )
                out.append(ins)
            if changed:
                bb.instructions = out

# --- problem constants (hardcoded per spec) ---------------------------------
B, H, W = 16, 1024, 1024
NUM_LABELS = 64
N_CORES = 8
BPC = B // N_CORES            # batches per core = 2
PX = H * W                    # pixels per batch = 1048576
P = 128
COLS = PX // P                # 8192 free-dim columns per batch
TILE_N = 4096
TPB = COLS // TILE_N          # tiles per batch = 2
NTILES = BPC * TPB            # tiles per core = 4

F32 = mybir.dt.float32
I32 = mybir.dt.int32
BF16 = mybir.dt.bfloat16
ALU = mybir.AluOpType

_CACHED_NC = None


def build_nc():
    global _CACHED_NC
    if _CACHED_NC is not None:
        return _CACHED_NC
    nc = bass.Bass()
    f_in = nc.dram_tensor("featmap", [BPC, P, COLS], F32, kind="ExternalInput")
    g_in = nc.dram_tensor("gt", [BPC, P, COLS], I32, kind="ExternalInput")
    out = nc.dram_tensor("out", [1, 1], F32, kind="ExternalOutput")

    with tile.TileContext(nc) as tc:
        with (
            tc.tile_pool(name="fin", bufs=2) as fin_pool,
            tc.tile_pool(name="gin", bufs=2) as gin_pool,
            tc.tile_pool(name="gf", bufs=2) as gf_pool,
            tc.tile_pool(name="gbf", bufs=2) as gbf_pool,
            tc.tile_pool(name="sq", bufs=2) as sq_pool,
            tc.tile_pool(name="dbf", bufs=2) as d_pool,
            tc.tile_pool(name="dum", bufs=1) as dum_pool,
            tc.tile_pool(name="acc", bufs=1) as acc_pool,
            tc.tile_pool(name="fini", bufs=1) as fini_pool,
        ):
            # per-(label, tile) accumulator columns: col = l*NTILES + t
            acc_s = acc_pool.tile([P, NUM_LABELS * NTILES], F32)
            acc_c = acc_pool.tile([P, NUM_LABELS * NTILES], F32)
            dummies = [dum_pool.tile([P, TILE_N], BF16, name=f"dm{i}", tag=f"dm{i}") for i in range(4)]
            ones_bf = dum_pool.tile([P, TILE_N], BF16, name="ones_bf")
            nc.vector.memset(ones_bf[:], 1.0)

            for t in range(NTILES):
                b, tb = divmod(t, TPB)
                csl = slice(tb * TILE_N, (tb + 1) * TILE_N)
                f_t = fin_pool.tile([P, TILE_N], F32)
                g_t = gin_pool.tile([P, TILE_N], I32)
                nc.gpsimd.dma_start(out=f_t[:], in_=f_in[b, :, csl])
                nc.gpsimd.dma_start(out=g_t[:], in_=g_in[b, :, csl])

                g_f = gf_pool.tile([P, TILE_N], F32)
                nc.vector.tensor_copy(g_f[:], g_t[:])
                g_bf = gbf_pool.tile([P, TILE_N], BF16)
                nc.vector.tensor_copy(g_bf[:], g_t[:])
                d_bf = d_pool.tile([P, TILE_N], BF16)
                nc.vector.tensor_tensor(
                    out=d_bf[:], in0=f_t[:], in1=g_f[:], op=ALU.subtract
                )
                sq = sq_pool.tile([P, TILE_N], BF16)
                nc.scalar.activation(
                    sq[:], d_bf[:], mybir.ActivationFunctionType.Square
                )

                for l in range(NUM_LABELS):
                    col = l * NTILES + t
                    nc.vector.scalar_tensor_tensor(
                        out=dummies[l % 4][:],
                        in0=g_bf[:],
                        scalar=float(l),
                        in1=sq[:],
                        op0=ALU.is_equal,
                        op1=ALU.mult,
                        accum_out=acc_s[:, col : col + 1],
                    )
                    nc.vector.tensor_scalar(
                        out=dummies[(l + 2) % 4][:],
                        in0=g_bf[:],
                        scalar1=float(l),
                        scalar2=0.0,
                        op0=ALU.is_equal,
                        op1=ALU.add,
                        accum_out=acc_c[:, col : col + 1],
                    )

            # ---- final reduction (tiny) ----
            # X-reduce tiles-per-batch: [128, 64, BPC, TPB] -> [128, 64*BPC]
            red_s = fini_pool.tile([P, NUM_LABELS * BPC], F32)
            red_c = fini_pool.tile([P, NUM_LABELS * BPC], F32)
            nc.vector.tensor_reduce(
                out=red_s[:],
                in_=acc_s[:].rearrange("p (l b t) -> p (l b) t", l=NUM_LABELS, b=BPC),
                axis=mybir.AxisListType.X,
                op=ALU.add,
            )
            nc.vector.tensor_reduce(
                out=red_c[:],
                in_=acc_c[:].rearrange("p (l b t) -> p (l b) t", l=NUM_LABELS, b=BPC),
                axis=mybir.AxisListType.X,
                op=ALU.add,
            )
            # partition reduce via ones-matmul on the Tensor engine
            nl0 = NUM_LABELS * BPC
            ones = fini_pool.tile([P, 1], F32)
            nc.vector.memset(ones[:], 1.0)
            with tc.tile_pool(name="ps", bufs=1, space="PSUM") as psum_pool:
                ps_s = psum_pool.tile([1, nl0], F32)
                ps_c = psum_pool.tile([1, nl0], F32)
                nc.tensor.matmul(ps_s[:], lhsT=ones[:], rhs=red_s[:], start=True, stop=True)
                nc.tensor.matmul(ps_c[:], lhsT=ones[:], rhs=red_c[:], start=True, stop=True)
                par_s = fini_pool.tile([1, nl0], F32)
                par_c = fini_pool.tile([1, nl0], F32)
                nc.vector.tensor_copy(par_s[:], ps_s[:])
                nc.vector.tensor_copy(par_c[:], ps_c[:])
            # scalar math on partition-0 row: [1, 128] with col = l*BPC + b
            nl = NUM_LABELS * BPC
            cclamp = fini_pool.tile([1, nl], F32)
            nc.vector.tensor_scalar(
                out=cclamp[:], in0=par_c[:, :], scalar1=1.0, scalar2=None, op0=ALU.max
            )
            inv = fini_pool.tile([1, nl], F32)
            nc.vector.reciprocal(inv[:], cclamp[:])
            contrib = fini_pool.tile([1, nl], F32)
            nc.vector.tensor_tensor(
                out=contrib[:], in0=par_s[:, :], in1=inv[:], op=ALU.mult
            )
            mask = fini_pool.tile([1, nl], F32)
            nc.vector.tensor_scalar(
                out=mask[:], in0=par_c[:, :], scalar1=0.5, scalar2=None, op0=ALU.is_ge
            )
            gated = fini_pool.tile([1, nl], F32)
            nc.vector.tensor_tensor(
                out=gated[:], in0=contrib[:], in1=mask[:], op=ALU.mult
            )
            # sum over labels 1..63, both batches: cols [BPC:] skip label 0
            loss = fini_pool.tile([1, 1], F32)
            nc.vector.tensor_reduce(
                out=loss[:],
                in_=gated[:, BPC:],
                axis=mybir.AxisListType.X,
                op=ALU.add,
            )
            nc.gpsimd.dma_start(out=out[:, :], in_=loss[:])
    _split_waits(nc)
    _CACHED_NC = nc
    return nc


def kernel(featmap: np.ndarray, gt: np.ndarray) -> np.ndarray:
    assert featmap.shape == (B, 1, H, W) and gt.shape == (B, 1, H, W)
    f = np.ascontiguousarray(featmap, dtype=np.float32).reshape(B, PX)
    g = np.ascontiguousarray(gt, dtype=np.int32).reshape(B, PX)
    nc = build_nc()
    in_maps = []
    for c in range(N_CORES):
        sl = slice(c * BPC, (c + 1) * BPC)
        in_maps.append(
            {
                "featmap": f[sl].reshape(BPC, P, COLS),
                "gt": g[sl].reshape(BPC, P, COLS),
            }
        )
    res = run_bass_kernel_spmd(nc, in_maps, core_ids=list(range(N_CORES)))
    total = sum(float(r["out"][0, 0]) for r in res.results)
    return np.float32(total / B)



# revision 7
# speedup vs baseline: 1.8175x; 1.8175x over previous
"""Trainium2 Bass kernel for nn_HSL1Loss (per-(batch,label) segment MSE loss).

loss = (1/B) * sum_b sum_{l=1..63, cnt>0} mean((feat[b][gt[b]==l] - l)^2)

Strategy: batch-data-parallel over 8 NeuronCores (2 images each). The wall
clock of a cached call is dominated by host->device transfer over the axon
tunnel (~75 MB/s), so the host packs both inputs into ONE uint8 tensor per
core: featmap truncated to bf16 (high 2 bytes of each f32) followed by gt as
uint8 (labels are 0..63) -- 3 bytes/pixel = 48 MB total vs 128 MB for
f32+int32. On device, each [128, N] tile is reduced into per-(batch,label)
sum/count accumulators with 64 fused mask-multiply-accumulate passes
(scalar_tensor_tensor with accum_out, bf16) plus 64 count passes
(tensor_scalar is_equal with accum_out) on the Vector engine. Squared error
is produced on the Scalar engine. Partition reduce via ones-matmul on the
Tensor engine, division + final reduction on-device; host sums the 8
per-core partials (the scalar all-reduce).
"""
import numpy as np

import concourse.bass as bass
import concourse.bass_isa as bass_isa
import concourse.mybir as mybir
import concourse.tile as tile
from concourse.bass_utils import run_bass_kernel_spmd

# --- inline tile drain patch (kernel.py must be self-contained) -------------
from concourse import tile as _tile_mod


def _apply_drain_patch(max_waits=1):
    if getattr(_tile_mod.TileContext, "_drain_split_patched", False):
        return

    def _drain_and_barrier(self, tick_clock, wait_clock):
        drain_inst = self.nc.sync.drain()
        wait_clock.add_sem_waits(
            drain_inst.ins, _tile_mod.ScopedClock({None: tick_clock.global_clock})
        )
        si = drain_inst.ins.sync_info
        waits = list(si.on_wait or []) if si is not None else []
        if len(waits) > max_waits:
            upd = list(si.on_update or [])
            drain_inst.ins.sync_info = mybir.SyncInfo(
                on_wait=waits[:max_waits], on_update=upd
            )
            for i in range(max_waits, len(waits), max_waits):
                d2 = self.nc.sync.drain()
                d2.ins.sync_info = mybir.SyncInfo(
                    on_wait=waits[i : i + max_waits], on_update=[]
                )
        self.nc.all_engine_barrier()
        assert self.sems is not None
        popped = self.nc._tile_sem_poison_stack.pop()
        assert popped is self._sem_poison
        self.nc.clear_and_free_semaphores(list(self.sems.allocated().values()))
        self.nc.all_engine_barrier()

    _tile_mod.TileContext._drain_and_barrier = _drain_and_barrier
    _tile_mod.TileContext._drain_split_patched = True


_apply_drain_patch()

_MAX_INST_WAITS = 1
_wsplit_counter = [0]


def _split_waits(nc, k=_MAX_INST_WAITS):
    """Walrus in this toolchain rejects instructions with >k sem waits.
    Move excess waits onto same-engine NoOps inserted just before."""
    for fn in nc.m.functions:
        for bb in fn.blocks:
            il = list(bb.instructions)
            out = []
            changed = False
            for ins in il:
                si = ins.sync_info
                waits = list(si.on_wait or []) if si is not None else []
                if len(waits) > k:
                    changed = True
                    chunks = [waits[i : i + k] for i in range(0, len(waits), k)]
                    for ch in chunks[:-1]:
                        _wsplit_counter[0] += 1
                        nop = mybir.InstNoOp(
                            name=f"WSPLIT-{_wsplit_counter[0]}", ins=[], outs=[]
                        )
                        nop.engine = ins.engine
                        nop.sync_info = mybir.SyncInfo(on_wait=ch, on_update=[])
                        out.append(nop)
                    ins.sync_info = mybir.SyncInfo(
                        on_wait=chunks[-1], on_update=list(si.on_update or [])
                    )
                out.append(ins)
            if changed:
                bb.instructions = out

# --- problem constants (hardcoded per spec) ---------------------------------
B, H, W = 16, 1024, 1024
NUM_LABELS = 64
N_CORES = 8
BPC = B // N_CORES            # batches per core = 2
PX = H * W                    # pixels per batch = 1048576
P = 128
COLS = PX // P                # 8192 free-dim columns per batch
TILE_N = 4096
TPB = COLS // TILE_N          # tiles per batch = 2
NTILES = BPC * TPB            # tiles per core = 4
ROWB = 3 * COLS               # packed bytes per (batch, partition) row

F32 = mybir.dt.float32
U8 = mybir.dt.uint8
BF16 = mybir.dt.bfloat16
ALU = mybir.AluOpType

_CACHED_NC = None


def build_nc():
    global _CACHED_NC
    if _CACHED_NC is not None:
        return _CACHED_NC
    nc = bass.Bass()
    # packed input: per (batch, partition) row, bytes [0 : 2*COLS) hold the
    # bf16 featmap row; bytes [2*COLS : 3*COLS) hold the uint8 gt row.
    fgt = nc.dram_tensor("fgt", [BPC, P, ROWB], U8, kind="ExternalInput")
    out = nc.dram_tensor("out", [1, 1], F32, kind="ExternalOutput")

    with tile.TileContext(nc) as tc:
        with (
            tc.tile_pool(name="fin", bufs=2) as fin_pool,
            tc.tile_pool(name="gin", bufs=2) as gin_pool,
            tc.tile_pool(name="gbf", bufs=2) as gbf_pool,
            tc.tile_pool(name="sq", bufs=2) as sq_pool,
            tc.tile_pool(name="dbf", bufs=2) as d_pool,
            tc.tile_pool(name="dum", bufs=1) as dum_pool,
            tc.tile_pool(name="acc", bufs=1) as acc_pool,
            tc.tile_pool(name="fini", bufs=1) as fini_pool,
        ):
            # per-(label, tile) accumulator columns: col = l*NTILES + t
            acc_s = acc_pool.tile([P, NUM_LABELS * NTILES], F32)
            acc_c = acc_pool.tile([P, NUM_LABELS * NTILES], F32)
            vdum = [dum_pool.tile([P, TILE_N], BF16, name=f"vd{i}", tag=f"vd{i}") for i in range(4)]

            for t in range(NTILES):
                b, tb = divmod(t, TPB)
                f_t = fin_pool.tile([P, TILE_N], BF16)
                nc.gpsimd.dma_start(
                    out=f_t[:],
                    in_=fgt[b, :, 2 * TILE_N * tb : 2 * TILE_N * (tb + 1)].bitcast(BF16),
                )
                g_t = gin_pool.tile([P, TILE_N], U8)
                nc.gpsimd.dma_start(
                    out=g_t[:],
                    in_=fgt[b, :, 2 * COLS + TILE_N * tb : 2 * COLS + TILE_N * (tb + 1)],
                )

                g_bf = gbf_pool.tile([P, TILE_N], BF16)
                nc.vector.tensor_copy(g_bf[:], g_t[:])
                d_bf = d_pool.tile([P, TILE_N], BF16)
                nc.vector.tensor_tensor(
                    out=d_bf[:], in0=f_t[:], in1=g_bf[:], op=ALU.subtract
                )
                sq = sq_pool.tile([P, TILE_N], BF16)
                nc.scalar.activation(
                    sq[:], d_bf[:], mybir.ActivationFunctionType.Square
                )

                for l in range(NUM_LABELS):
                    col = l * NTILES + t
                    nc.vector.scalar_tensor_tensor(
                        out=vdum[l % 4][:],
                        in0=g_bf[:],
                        scalar=float(l),
                        in1=sq[:],
                        op0=ALU.is_equal,
                        op1=ALU.mult,
                        accum_out=acc_s[:, col : col + 1],
                    )
                for l in range(NUM_LABELS):
                    col = l * NTILES + t
                    nc.vector.tensor_scalar(
                        out=vdum[(l + 2) % 4][:],
                        in0=g_bf[:],
                        scalar1=float(l),
                        scalar2=0.0,
                        op0=ALU.is_equal,
                        op1=ALU.add,
                        accum_out=acc_c[:, col : col + 1],
                    )

            # ---- final reduction (tiny) ----
            # X-reduce tiles-per-batch: [128, l, BPC, TPB] -> [128, l*BPC]
            red_s = fini_pool.tile([P, NUM_LABELS * BPC], F32)
            red_c = fini_pool.tile([P, NUM_LABELS * BPC], F32)
            nc.vector.tensor_reduce(
                out=red_s[:],
                in_=acc_s[:].rearrange("p (l b t) -> p (l b) t", l=NUM_LABELS, b=BPC),
                axis=mybir.AxisListType.X,
                op=ALU.add,
            )
            nc.vector.tensor_reduce(
                out=red_c[:],
                in_=acc_c[:].rearrange("p (l b t) -> p (l b) t", l=NUM_LABELS, b=BPC),
                axis=mybir.AxisListType.X,
                op=ALU.add,
            )
            # partition reduce via ones-matmul on the Tensor engine
            nl = NUM_LABELS * BPC
            ones = fini_pool.tile([P, 1], F32)
            nc.vector.memset(ones[:], 1.0)
            with tc.tile_pool(name="ps", bufs=1, space="PSUM") as psum_pool:
                ps_s = psum_pool.tile([1, nl], F32)
                ps_c = psum_pool.tile([1, nl], F32)
                nc.tensor.matmul(ps_s[:], lhsT=ones[:], rhs=red_s[:], start=True, stop=True)
                nc.tensor.matmul(ps_c[:], lhsT=ones[:], rhs=red_c[:], start=True, stop=True)
                par_s = fini_pool.tile([1, nl], F32)
                par_c = fini_pool.tile([1, nl], F32)
                nc.vector.tensor_copy(par_s[:], ps_s[:])
                nc.vector.tensor_copy(par_c[:], ps_c[:])
            # scalar math on partition-0 row: [1, nl] with col = l*BPC + b
            cclamp = fini_pool.tile([1, nl], F32)
            nc.vector.tensor_scalar(
                out=cclamp[:], in0=par_c[:, :], scalar1=1.0, scalar2=None, op0=ALU.max
            )
            inv = fini_pool.tile([1, nl], F32)
            nc.vector.reciprocal(inv[:], cclamp[:])
            contrib = fini_pool.tile([1, nl], F32)
            nc.vector.tensor_tensor(
                out=contrib[:], in0=par_s[:, :], in1=inv[:], op=ALU.mult
            )
            mask = fini_pool.tile([1, nl], F32)
            nc.vector.tensor_scalar(
                out=mask[:], in0=par_c[:, :], scalar1=0.5, scalar2=None, op0=ALU.is_ge
            )
            gated = fini_pool.tile([1, nl], F32)
            nc.vector.tensor_tensor(
                out=gated[:], in0=contrib[:], in1=mask[:], op=ALU.mult
            )
            # sum over labels 1..63, both batches: cols [BPC:] skip label 0
            loss = fini_pool.tile([1, 1], F32)
            nc.vector.tensor_reduce(
                out=loss[:],
                in_=gated[:, BPC:],
                axis=mybir.AxisListType.X,
                op=ALU.add,
            )
            nc.gpsimd.dma_start(out=out[:, :], in_=loss[:])
    _split_waits(nc)
    _CACHED_NC = nc
    return nc


def _pack_inputs(featmap: np.ndarray, gt: np.ndarray) -> np.ndarray:
    """Pack f32 featmap (as truncated bf16 bytes) + gt (as uint8) into one
    [B, P, 3*COLS] uint8 array. Little-endian: the bf16 of an f32 is its
    high uint16."""
    f = np.ascontiguousarray(featmap, dtype=np.float32).reshape(B, PX)
    g = np.asarray(gt).reshape(B, P, COLS)
    buf = np.empty((B, P, ROWB), np.uint8)
    buf[:, :, : 2 * COLS].view(np.uint16)[...] = (
        f.view(np.uint16)[:, 1::2].reshape(B, P, COLS)
    )
    buf[:, :, 2 * COLS :] = g  # int32 -> uint8 (values 0..63)
    return buf


def kernel(featmap: np.ndarray, gt: np.ndarray) -> np.ndarray:
    assert featmap.shape == (B, 1, H, W) and gt.shape == (B, 1, H, W)
    buf = _pack_inputs(featmap, gt)
    nc = build_nc()
    in_maps = [{"fgt": buf[c * BPC : (c + 1) * BPC]} for c in range(N_CORES)]
    res = run_bass_kernel_spmd(nc, in_maps, core_ids=list(range(N_CORES)))
    total = sum(float(r["out"][0, 0]) for r in res.results)
    return np.float32(total / B)
